# revision 8
# baseline (speedup 1.0000x reference)
"""Causal self-attention (GQA + RoPE) TRN2 Bass kernel, 8-way sharded.

Sharding: core c -> batch b = c//4, KV group g = c%4 (Q heads 4g..4g+3).
Each core computes its 4 Q heads' attention for its batch plus the partial
output projection (rows of Wo for those heads). Host sums the 4 partials
per batch and transposes back to [S, H].

Device layouts are transposed ([feature, seq]) so every matmul contracts
along the partition dim. All matmuls run in float32r (TF32) at full PE
rate; softmax runs in fp32 without max-subtraction (logits are O(5) for
this distribution, exp is safe).
"""
import sys
sys.path.insert(0, '/opt/trn_rl_repo')
import numpy as np

import concourse.bacc as bacc
import concourse.tile as tile
from concourse import mybir
from concourse.bass_utils import run_bass_kernel_spmd

F32 = mybir.dt.float32
F32R = mybir.dt.float32r

B, S, H = 2, 2048, 2048
NH, NKV, HD = 16, 4, 128
ROPE_BASE = 10000.0
SCALE = 1.0 / float(np.sqrt(HD))
NCORES = 8
HEADS_PER_CORE = NH // NKV          # 4 Q heads per KV group
DLOC = HEADS_PER_CORE * HD          # 512 local head dims
P = 128                             # partitions
NQC = S // 512                      # 4 query chunks of 512
NKB = S // P                        # 16 key blocks of 128
NHT = H // P                        # 16 contraction tiles of 128
NEG = -1.0e30

_CACHED_NC = None


def _build_program():
    """Emit the SPMD per-core program (identical on all 8 cores)."""
    nc = bacc.Bacc("TRN2", target_bir_lowering=False, debug=False)

    d_xT = nc.dram_tensor("xT", [H, S], F32R, kind="ExternalInput").ap()
    d_wq = nc.dram_tensor("wq", [H, DLOC], F32R, kind="ExternalInput").ap()
    d_wk = nc.dram_tensor("wk", [H, HD], F32R, kind="ExternalInput").ap()
    d_wv = nc.dram_tensor("wv", [H, HD], F32R, kind="ExternalInput").ap()
    d_wo = nc.dram_tensor("wo", [DLOC, H], F32R, kind="ExternalInput").ap()
    d_cos = nc.dram_tensor("cos", [P, S], F32, kind="ExternalInput").ap()
    d_sin = nc.dram_tensor("sin", [P, S], F32, kind="ExternalInput").ap()
    d_perm = nc.dram_tensor("perm", [P, P], F32R, kind="ExternalInput").ap()
    d_ones = nc.dram_tensor("ones", [P, 1], F32R, kind="ExternalInput").ap()
    d_mask = nc.dram_tensor("maskneg", [P, P], F32, kind="ExternalInput").ap()
    d_id = nc.dram_tensor("ident", [P, P], F32R, kind="ExternalInput").ap()
    d_yT = nc.dram_tensor("yT", [H, S], F32, kind="ExternalOutput").ap()

    with tile.TileContext(nc) as tc:
        with tc.tile_pool(name="persist", bufs=1) as pers:
            # Persistent SBUF tensors: roped Q^T/K^T, V ([s,d] blocks),
            # unnormalized-then-normalized attention output A.
            qT = [pers.tile([P, S], F32R, tag=f"qT{h}", name=f"qT{h}") for h in range(HEADS_PER_CORE)]
            kT = pers.tile([P, S], F32R, tag="kT")
            vblk = [pers.tile([P, HD], F32R, tag=f"v{sb}", name=f"v{sb}") for sb in range(NKB)]
            aT = [pers.tile([P, S], F32R, tag=f"aT{h}", name=f"aT{h}") for h in range(HEADS_PER_CORE)]

            # ---------------- Phase 1: projections + RoPE -----------------
            with tc.tile_pool(name="p1sb", bufs=1) as p1, \
                 tc.tile_pool(name="p1x", bufs=2) as p1x, \
                 tc.tile_pool(name="p1w", bufs=4) as p1w, \
                 tc.tile_pool(name="p1st", bufs=2) as p1s, \
                 tc.tile_pool(name="p1ps", bufs=1, space="PSUM") as pp, \
                 tc.tile_pool(name="p1ps2", bufs=1, space="PSUM") as pp2:
                t_cos = p1.tile([P, S], F32, tag="cos")
                t_sin = p1.tile([P, S], F32, tag="sin")
                t_perm = p1.tile([P, P], F32R, tag="perm")
                t_id = p1.tile([P, P], F32R, tag="ident")
                nc.sync.dma_start(out=t_cos[:], in_=d_cos)
                nc.sync.dma_start(out=t_sin[:], in_=d_sin)
                nc.sync.dma_start(out=t_perm[:], in_=d_perm)
                nc.sync.dma_start(out=t_id[:], in_=d_id)

                for qt in range(4):               # s quarters of 512
                    s0 = qt * 512
                    xts = []
                    for ht in range(NHT):
                        xt = p1x.tile([P, 512], F32R, tag=f"xt{ht}")
                        nc.sync.dma_start(out=xt[:],
                                          in_=d_xT[ht * P:(ht + 1) * P, s0:s0 + 512])
                        xts.append(xt)
                    # 6 concurrent PSUM accumulators: 4 Q chunks, K, V
                    accs = [pp.tile([P, 512], F32, tag=f"acc{c}", name=f"pacc{c}") for c in range(6)]
                    for ht in range(NHT):
                        wqt = p1w.tile([P, DLOC], F32R, tag="wqt")
                        nc.sync.dma_start(out=wqt[:],
                                          in_=d_wq[ht * P:(ht + 1) * P, :])
                        wkt = p1w.tile([P, HD], F32R, tag="wkt")
                        nc.sync.dma_start(out=wkt[:],
                                          in_=d_wk[ht * P:(ht + 1) * P, :])
                        wvt = p1w.tile([P, HD], F32R, tag="wvt")
                        nc.sync.dma_start(out=wvt[:],
                                          in_=d_wv[ht * P:(ht + 1) * P, :])
                        st = (ht == 0)
                        sp = (ht == NHT - 1)
                        for c in range(HEADS_PER_CORE):
                            nc.tensor.matmul(accs[c][:],
                                             wqt[:, c * P:(c + 1) * P], xts[ht][:],
                                             start=st, stop=sp)
                        nc.tensor.matmul(accs[4][:], wkt[:], xts[ht][:],
                                         start=st, stop=sp)
                        nc.tensor.matmul(accs[5][:], wvt[:], xts[ht][:],
                                         start=st, stop=sp)

                    # RoPE for the 4 Q chunks and K chunk
                    for c in range(5):
                        dst = qT[c] if c < HEADS_PER_CORE else kT
                        pre = p1s.tile([P, 512], F32R, tag="pre")
                        nc.scalar.copy(out=pre[:], in_=accs[c][:])
                        rot = pp2.tile([P, 512], F32, tag="rot")
                        nc.tensor.matmul(rot[:], t_perm[:], pre[:],
                                         start=True, stop=True)
                        t1 = p1s.tile([P, 512], F32, tag="t1")
                        nc.vector.tensor_mul(t1[:], pre[:].bitcast(F32),
                                             t_cos[:, s0:s0 + 512])
                        t2 = p1s.tile([P, 512], F32, tag="t2")
                        nc.vector.tensor_mul(t2[:], rot[:], t_sin[:, s0:s0 + 512])
                        nc.vector.tensor_add(dst[:, s0:s0 + 512], t1[:], t2[:])

                    # V: copy out then PE-transpose into [s, d] blocks
                    vt_s = p1s.tile([P, 512], F32R, tag="vts")
                    nc.scalar.copy(out=vt_s[:], in_=accs[5][:])
                    for j in range(4):
                        ps_vt = pp2.tile([P, P], F32R, tag="psvt")
                        nc.tensor.transpose(ps_vt[:], vt_s[:, j * P:(j + 1) * P],
                                            t_id[:])
                        nc.scalar.copy(out=vblk[qt * 4 + j][:],
                                       in_=ps_vt[:].bitcast(F32))

            # ---------------- Phase 2: causal attention -------------------
            with tc.tile_pool(name="p2sb", bufs=1) as p2, \
                 tc.tile_pool(name="p2pt", bufs=6) as p2pt, \
                 tc.tile_pool(name="p2acc", bufs=2) as p2a, \
                 tc.tile_pool(name="p2ps", bufs=3, space="PSUM") as psc, \
                 tc.tile_pool(name="p2pat", bufs=2, space="PSUM") as pat, \
                 tc.tile_pool(name="p2psum", bufs=2, space="PSUM") as psums:
                t_mask = p2.tile([P, P], F32, tag="mask")
                t_ones = p2.tile([P, 1], F32R, tag="ones")
                nc.sync.dma_start(out=t_mask[:], in_=d_mask)
                nc.sync.dma_start(out=t_ones[:], in_=d_ones)

                for h in range(HEADS_PER_CORE):
                    for qc in range(NQC):
                        q0 = qc * 512
                        nkb = 4 * qc + 4          # key blocks 0..nkb-1
                        attn = pat.tile([P, 512], F32, tag="attn")
                        acc = p2a.tile([P, 512], F32, tag="sumacc")
                        for kb in range(nkb):
                            j = kb - 4 * qc       # >=0 on the diagonal
                            off = j * P if j >= 0 else 0
                            w = 512 - off
                            ps_s = psc.tile([P, 512], F32, tag="ps_s")
                            nc.tensor.matmul(ps_s[:, :w],
                                             kT[:, kb * P:(kb + 1) * P],
                                             qT[h][:, q0 + off:q0 + 512],
                                             start=True, stop=True)
                            if j >= 0:
                                nc.vector.tensor_add(ps_s[:, 0:P], ps_s[:, 0:P],
                                                     t_mask[:])
                            pt = p2pt.tile([P, 512], F32R, tag="pt")
                            nc.scalar.activation(
                                out=pt[:, :w], in_=ps_s[:, :w],
                                func=mybir.ActivationFunctionType.Exp, scale=SCALE)
                            nc.tensor.matmul(attn[:, off:512], vblk[kb][:],
                                             pt[:, :w],
                                             start=(kb == 0), stop=(kb == nkb - 1))
                            if kb == 0:
                                nc.vector.tensor_copy(out=acc[:],
                                                      in_=pt[:].bitcast(F32))
                            else:
                                nc.vector.tensor_add(acc[:, off:512],
                                                     acc[:, off:512],
                                                     pt[:, :w].bitcast(F32))
                        accr = p2a.tile([P, 512], F32R, tag="accr")
                        nc.scalar.copy(out=accr[:], in_=acc[:])
                        ps_sm = psums.tile([1, 512], F32, tag="ps_sm")
                        nc.tensor.matmul(ps_sm[:], t_ones[:], accr[:],
                                         start=True, stop=True)
                        rec = p2a.tile([1, 512], F32, tag="rec")
                        nc.vector.reciprocal(out=rec[:], in_=ps_sm[:])
                        recb = p2a.tile([P, 512], F32, tag="recb")
                        nc.gpsimd.partition_broadcast(recb[:], rec[:])
                        nc.vector.tensor_mul(aT[h][:, q0:q0 + 512], attn[:],
                                             recb[:])

            # ---------------- Phase 3: output projection ------------------
            with tc.tile_pool(name="p3sb", bufs=1) as p3, \
                 tc.tile_pool(name="p3y", bufs=3) as p3y, \
                 tc.tile_pool(name="p3ps", bufs=4, space="PSUM") as pyo:
                wot = []
                for dinb in range(HEADS_PER_CORE):
                    wt = p3.tile([P, H], F32R, tag=f"wo{dinb}")
                    nc.sync.dma_start(out=wt[:],
                                      in_=d_wo[dinb * P:(dinb + 1) * P, :])
                    wot.append(wt)
                for hc in range(NHT):
                    yt_sb = p3y.tile([P, S], F32, tag="yt")
                    for nch in range(NQC):
                        n0 = nch * 512
                        accy = pyo.tile([P, 512], F32, tag="accy")
                        for dinb in range(HEADS_PER_CORE):
                            nc.tensor.matmul(accy[:],
                                             wot[dinb][:, hc * P:(hc + 1) * P],
                                             aT[dinb][:, n0:n0 + 512],
                                             start=(dinb == 0),
                                             stop=(dinb == HEADS_PER_CORE - 1))
                        nc.scalar.copy(out=yt_sb[:, n0:n0 + 512], in_=accy[:])
                    nc.sync.dma_start(out=d_yT[hc * P:(hc + 1) * P, :],
                                      in_=yt_sb[:])
    nc.compile()
    return nc


def _get_program():
    global _CACHED_NC
    if _CACHED_NC is None:
        _CACHED_NC = _build_program()
    return _CACHED_NC


def _host_tables(position_ids):
    pos = np.asarray(position_ids).reshape(-1).astype(np.float64)  # [S]
    half = HD // 2
    inv = 1.0 / (ROPE_BASE ** (np.arange(half, dtype=np.float64) * 2.0 / HD))
    ang = pos[:, None] * inv[None, :]                 # [S, 64]
    cos = np.cos(ang).T.astype(np.float32)            # [64, S]
    sin = np.sin(ang).T.astype(np.float32)
    cos_t = np.ascontiguousarray(np.concatenate([cos, cos], axis=0))  # [128, S]
    sin_t = np.ascontiguousarray(np.concatenate([sin, sin], axis=0))
    # perm implements rotate_half in [d, s] layout, signs folded in:
    # out[d'] = -in[d'+64] (d'<64), +in[d'-64] (d'>=64); lhsT[d, d'] layout.
    perm = np.zeros((P, P), np.float32)
    for dp in range(half):
        perm[dp + half, dp] = -1.0
    for dp in range(half, HD):
        perm[dp - half, dp] = 1.0
    ones = np.ones((P, 1), np.float32)
    # additive causal mask for a diagonal block in [k, q] layout
    mask = np.where(np.arange(P)[:, None] > np.arange(P)[None, :], NEG,
                    0.0).astype(np.float32)
    ident = np.eye(P, dtype=np.float32)
    return cos_t, sin_t, perm, ones, mask, ident


def kernel(x, position_ids, Wq, Wk, Wv, Wo):
    x = np.asarray(x, dtype=np.float32)
    Wq = np.asarray(Wq, dtype=np.float32)
    Wk = np.asarray(Wk, dtype=np.float32)
    Wv = np.asarray(Wv, dtype=np.float32)
    Wo = np.asarray(Wo, dtype=np.float32)
    cos_t, sin_t, perm, ones, mask, ident = _host_tables(position_ids)

    in_maps = []
    for c in range(NCORES):
        b, g = c // NKV, c % NKV
        in_maps.append({
            "xT": np.ascontiguousarray(x[b].T),
            "wq": np.ascontiguousarray(Wq[:, g * DLOC:(g + 1) * DLOC]),
            "wk": np.ascontiguousarray(Wk[:, g * HD:(g + 1) * HD]),
            "wv": np.ascontiguousarray(Wv[:, g * HD:(g + 1) * HD]),
            "wo": np.ascontiguousarray(Wo[g * DLOC:(g + 1) * DLOC, :]),
            "cos": cos_t, "sin": sin_t, "perm": perm, "ones": ones,
            "maskneg": mask, "ident": ident,
        })

    nc = _get_program()
    res = run_bass_kernel_spmd(nc, in_maps, core_ids=list(range(NCORES)))

    out = np.zeros((B, S, H), np.float32)
    for c in range(NCORES):
        out[c // NKV] += res.results[c]["yT"].T
    return out


# revision 13
# speedup vs baseline: 1.1474x; 1.1474x over previous
"""Causal self-attention (GQA + RoPE) TRN2 Bass kernel, 8-way sharded.

Sharding: core c -> batch b = c//4, KV group g = c%4 (Q heads 4g..4g+3).
Each core computes its 4 Q heads' attention for its batch plus the partial
output projection (rows of Wo for those heads). Host sums the 4 partials
per batch and transposes back to [S, H].

Device layouts are transposed ([feature, seq]) so every matmul contracts
along the partition dim. All matmuls run in float32r (TF32) at full PE
rate; softmax runs in fp32 without max-subtraction (logits are O(5) for
this distribution, exp is safe).
"""
import sys
sys.path.insert(0, '/opt/trn_rl_repo')
import numpy as np

import concourse.bacc as bacc
import concourse.tile as tile
from concourse import mybir
from concourse.bass_utils import run_bass_kernel_spmd

F32 = mybir.dt.float32
F32R = mybir.dt.float32r

B, S, H = 2, 2048, 2048
NH, NKV, HD = 16, 4, 128
ROPE_BASE = 10000.0
SCALE = 1.0 / float(np.sqrt(HD))
NCORES = 8
HEADS_PER_CORE = NH // NKV          # 4 Q heads per KV group
DLOC = HEADS_PER_CORE * HD          # 512 local head dims
P = 128                             # partitions
NQC = S // 512                      # 4 query chunks of 512
NKB = S // P                        # 16 key blocks of 128
NHT = H // P                        # 16 contraction tiles of 128
NEG = -1.0e30

_CACHED_NC = None


def _build_program():
    """Emit the SPMD per-core program (identical on all 8 cores)."""
    nc = bacc.Bacc("TRN2", target_bir_lowering=False, debug=False)

    d_xT = nc.dram_tensor("xT", [H, S], F32R, kind="ExternalInput").ap()
    d_wq = nc.dram_tensor("wq", [H, DLOC], F32R, kind="ExternalInput").ap()
    d_wk = nc.dram_tensor("wk", [H, HD], F32R, kind="ExternalInput").ap()
    d_wv = nc.dram_tensor("wv", [H, HD], F32R, kind="ExternalInput").ap()
    d_wo = nc.dram_tensor("wo", [DLOC, H], F32R, kind="ExternalInput").ap()
    d_cos = nc.dram_tensor("cos", [P, S], F32, kind="ExternalInput").ap()
    d_sin = nc.dram_tensor("sin", [P, S], F32, kind="ExternalInput").ap()
    d_perm = nc.dram_tensor("perm", [P, P], F32R, kind="ExternalInput").ap()
    d_ones = nc.dram_tensor("ones", [P, 1], F32R, kind="ExternalInput").ap()
    d_mask = nc.dram_tensor("maskneg", [P, P], F32, kind="ExternalInput").ap()
    d_id = nc.dram_tensor("ident", [P, P], F32R, kind="ExternalInput").ap()
    d_yT = nc.dram_tensor("yT", [H, S], F32, kind="ExternalOutput").ap()

    with tile.TileContext(nc) as tc:
        with tc.tile_pool(name="persist", bufs=1) as pers:
            # Persistent SBUF tensors: roped Q^T/K^T, V ([s,d] blocks),
            # unnormalized-then-normalized attention output A.
            qT = [pers.tile([P, S], F32R, tag=f"qT{h}", name=f"qT{h}") for h in range(HEADS_PER_CORE)]
            kT = pers.tile([P, S], F32R, tag="kT")
            vblk = [pers.tile([P, HD], F32R, tag=f"v{sb}", name=f"v{sb}") for sb in range(NKB)]
            aT = [pers.tile([P, S], F32R, tag=f"aT{h}", name=f"aT{h}") for h in range(HEADS_PER_CORE)]

            # ---------------- Phase 1: projections + RoPE -----------------
            with tc.tile_pool(name="p1sb", bufs=1) as p1, \
                 tc.tile_pool(name="p1x", bufs=2) as p1x, \
                 tc.tile_pool(name="p1w", bufs=4) as p1w, \
                 tc.tile_pool(name="p1st", bufs=2) as p1s, \
                 tc.tile_pool(name="p1ps", bufs=1, space="PSUM") as pp, \
                 tc.tile_pool(name="p1ps2", bufs=1, space="PSUM") as pp2:
                t_cos = p1.tile([P, S], F32, tag="cos")
                t_sin = p1.tile([P, S], F32, tag="sin")
                t_perm = p1.tile([P, P], F32R, tag="perm")
                t_id = p1.tile([P, P], F32R, tag="ident")
                nc.sync.dma_start(out=t_cos[:], in_=d_cos)
                nc.sync.dma_start(out=t_sin[:], in_=d_sin)
                nc.sync.dma_start(out=t_perm[:], in_=d_perm)
                nc.sync.dma_start(out=t_id[:], in_=d_id)

                for qt in range(4):               # s quarters of 512
                    s0 = qt * 512
                    # 6 concurrent PSUM accumulators: 4 Q chunks, K, V
                    accs = [pp.tile([P, 512], F32, tag=f"acc{c}", name=f"pacc{c}") for c in range(6)]
                    for ht in range(NHT):
                        # interleave input DMAs with compute so the first
                        # matmul starts after ~2 DMAs, not a full-quarter fill
                        xt = p1x.tile([P, 512], F32R, tag="xt", bufs=6)
                        nc.sync.dma_start(out=xt[:],
                                          in_=d_xT[ht * P:(ht + 1) * P, s0:s0 + 512])
                        wqt = p1w.tile([P, DLOC], F32R, tag="wqt")
                        nc.sync.dma_start(out=wqt[:],
                                          in_=d_wq[ht * P:(ht + 1) * P, :])
                        wkt = p1w.tile([P, HD], F32R, tag="wkt")
                        nc.sync.dma_start(out=wkt[:],
                                          in_=d_wk[ht * P:(ht + 1) * P, :])
                        wvt = p1w.tile([P, HD], F32R, tag="wvt")
                        nc.sync.dma_start(out=wvt[:],
                                          in_=d_wv[ht * P:(ht + 1) * P, :])
                        st = (ht == 0)
                        sp = (ht == NHT - 1)
                        for c in range(HEADS_PER_CORE):
                            nc.tensor.matmul(accs[c][:],
                                             wqt[:, c * P:(c + 1) * P], xt[:],
                                             start=st, stop=sp)
                        nc.tensor.matmul(accs[4][:], wkt[:], xt[:],
                                         start=st, stop=sp)
                        nc.tensor.matmul(accs[5][:], wvt[:], xt[:],
                                         start=st, stop=sp)

                    # RoPE for the 4 Q chunks and K chunk
                    for c in range(5):
                        dst = qT[c] if c < HEADS_PER_CORE else kT
                        pre = p1s.tile([P, 512], F32R, tag="pre")
                        nc.scalar.copy(out=pre[:], in_=accs[c][:])
                        rot = pp2.tile([P, 512], F32, tag="rot")
                        nc.tensor.matmul(rot[:], t_perm[:], pre[:],
                                         start=True, stop=True)
                        t1 = p1s.tile([P, 512], F32, tag="t1")
                        nc.vector.tensor_mul(t1[:], pre[:].bitcast(F32),
                                             t_cos[:, s0:s0 + 512])
                        t2 = p1s.tile([P, 512], F32, tag="t2")
                        nc.vector.tensor_mul(t2[:], rot[:], t_sin[:, s0:s0 + 512])
                        nc.vector.tensor_add(dst[:, s0:s0 + 512], t1[:], t2[:])

                    # V: copy out then PE-transpose into [s, d] blocks
                    vt_s = p1s.tile([P, 512], F32R, tag="vts")
                    nc.scalar.copy(out=vt_s[:], in_=accs[5][:])
                    for j in range(4):
                        ps_vt = pp2.tile([P, P], F32R, tag="psvt")
                        nc.tensor.transpose(ps_vt[:], vt_s[:, j * P:(j + 1) * P],
                                            t_id[:])
                        nc.scalar.copy(out=vblk[qt * 4 + j][:],
                                       in_=ps_vt[:].bitcast(F32))

            # Prefetch phase-3 weights early so the output projection never
            # waits on DMA (8 MB takes ~25 us).
            wot = []
            with tc.tile_pool(name="p3w", bufs=1) as p3w:
                for dinb in range(HEADS_PER_CORE):
                    wt = p3w.tile([P, H], F32R, tag=f"wo{dinb}", name=f"wo{dinb}")
                    nc.sync.dma_start(out=wt[:],
                                      in_=d_wo[dinb * P:(dinb + 1) * P, :])
                    wot.append(wt)

                # -------------- Phase 2: causal attention -----------------
                # Two heads interleaved per iteration: two independent
                # PSUM accumulation chains keep the PE busy while ACT runs
                # the other head's exp.
                with tc.tile_pool(name="p2sb", bufs=1) as p2, \
                     tc.tile_pool(name="p2pt", bufs=8) as p2pt, \
                     tc.tile_pool(name="p2acc", bufs=2) as p2a, \
                     tc.tile_pool(name="p2ps", bufs=4, space="PSUM") as psc, \
                     tc.tile_pool(name="p2pat", bufs=1, space="PSUM") as pat, \
                     tc.tile_pool(name="p2psum", bufs=1, space="PSUM") as psums:
                    t_mask = p2.tile([P, P], F32, tag="mask")
                    t_ones = p2.tile([P, 1], F32R, tag="ones")
                    nc.sync.dma_start(out=t_mask[:], in_=d_mask)
                    nc.sync.dma_start(out=t_ones[:], in_=d_ones)

                    for hp in range(HEADS_PER_CORE // 2):
                        heads = (2 * hp, 2 * hp + 1)
                        for qc in range(NQC):
                            q0 = qc * 512
                            nkb = 4 * qc + 4      # key blocks 0..nkb-1
                            attn = [pat.tile([P, 512], F32, tag=f"attn{i}",
                                             name=f"attn{i}_{hp}_{qc}")
                                    for i in range(2)]
                            acc = [p2a.tile([P, 512], F32, tag=f"sumacc{i}",
                                            name=f"sumacc{i}_{hp}_{qc}")
                                   for i in range(2)]
                            for kb in range(nkb):
                                j = kb - 4 * qc   # >=0 on the diagonal
                                off = j * P if j >= 0 else 0
                                w = 512 - off
                                pts = []
                                for i, h in enumerate(heads):
                                    ps_s = psc.tile([P, 512], F32, tag="ps_s",
                                                    name=f"ps_{hp}_{qc}_{kb}_{i}")
                                    nc.tensor.matmul(ps_s[:, :w],
                                                     kT[:, kb * P:(kb + 1) * P],
                                                     qT[h][:, q0 + off:q0 + 512],
                                                     start=True, stop=True)
                                    if j >= 0:
                                        nc.vector.tensor_add(ps_s[:, 0:P],
                                                             ps_s[:, 0:P],
                                                             t_mask[:])
                                    pt = p2pt.tile([P, 512], F32R, tag="pt",
                                                   name=f"pt_{hp}_{qc}_{kb}_{i}")
                                    nc.scalar.activation(
                                        out=pt[:, :w], in_=ps_s[:, :w],
                                        func=mybir.ActivationFunctionType.Exp,
                                        scale=SCALE)
                                    pts.append(pt)
                                for i in range(2):
                                    nc.tensor.matmul(attn[i][:, off:512],
                                                     vblk[kb][:], pts[i][:, :w],
                                                     start=(kb == 0),
                                                     stop=(kb == nkb - 1))
                                for i in range(2):
                                    if kb == 0:
                                        nc.vector.tensor_copy(
                                            out=acc[i][:],
                                            in_=pts[i][:].bitcast(F32))
                                    else:
                                        nc.vector.tensor_add(
                                            acc[i][:, off:512],
                                            acc[i][:, off:512],
                                            pts[i][:, :w].bitcast(F32))
                            for i, h in enumerate(heads):
                                accr = p2a.tile([P, 512], F32R, tag="accr",
                                                name=f"accr_{hp}_{qc}_{i}")
                                nc.scalar.copy(out=accr[:], in_=acc[i][:])
                                ps_sm = psums.tile([1, 512], F32, tag=f"ps_sm{i}",
                                                   name=f"ps_sm_{hp}_{qc}_{i}")
                                nc.tensor.matmul(ps_sm[:], t_ones[:], accr[:],
                                                 start=True, stop=True)
                                rec = p2a.tile([1, 512], F32, tag="rec",
                                               name=f"rec_{hp}_{qc}_{i}")
                                rscr = p2a.tile([1, 512], F32, tag="rscr",
                                                name=f"rscr_{hp}_{qc}_{i}")
                                nc.vector.reciprocal_approx_accurate(
                                    out=rec[:], in_=ps_sm[:], scratch=rscr[:])
                                recb = p2a.tile([P, 512], F32, tag="recb",
                                                name=f"recb_{hp}_{qc}_{i}")
                                nc.gpsimd.partition_broadcast(recb[:], rec[:])
                                nc.vector.tensor_mul(aT[h][:, q0:q0 + 512],
                                                     attn[i][:], recb[:])

                # -------------- Phase 3: output projection ----------------
                with tc.tile_pool(name="p3y", bufs=3) as p3y, \
                     tc.tile_pool(name="p3ps", bufs=4, space="PSUM") as pyo:
                    for hc in range(NHT):
                        yt_sb = p3y.tile([P, S], F32, tag="yt")
                        for nch in range(NQC):
                            n0 = nch * 512
                            accy = pyo.tile([P, 512], F32, tag="accy")
                            for dinb in range(HEADS_PER_CORE):
                                nc.tensor.matmul(accy[:],
                                                 wot[dinb][:, hc * P:(hc + 1) * P],
                                                 aT[dinb][:, n0:n0 + 512],
                                                 start=(dinb == 0),
                                                 stop=(dinb == HEADS_PER_CORE - 1))
                            nc.scalar.copy(out=yt_sb[:, n0:n0 + 512], in_=accy[:])
                        nc.sync.dma_start(out=d_yT[hc * P:(hc + 1) * P, :],
                                          in_=yt_sb[:])
    nc.compile()
    return nc


def _get_program():
    global _CACHED_NC
    if _CACHED_NC is None:
        _CACHED_NC = _build_program()
    return _CACHED_NC


def _host_tables(position_ids):
    pos = np.asarray(position_ids).reshape(-1).astype(np.float64)  # [S]
    half = HD // 2
    inv = 1.0 / (ROPE_BASE ** (np.arange(half, dtype=np.float64) * 2.0 / HD))
    ang = pos[:, None] * inv[None, :]                 # [S, 64]
    cos = np.cos(ang).T.astype(np.float32)            # [64, S]
    sin = np.sin(ang).T.astype(np.float32)
    cos_t = np.ascontiguousarray(np.concatenate([cos, cos], axis=0))  # [128, S]
    sin_t = np.ascontiguousarray(np.concatenate([sin, sin], axis=0))
    # perm implements rotate_half in [d, s] layout, signs folded in:
    # out[d'] = -in[d'+64] (d'<64), +in[d'-64] (d'>=64); lhsT[d, d'] layout.
    perm = np.zeros((P, P), np.float32)
    for dp in range(half):
        perm[dp + half, dp] = -1.0
    for dp in range(half, HD):
        perm[dp - half, dp] = 1.0
    ones = np.ones((P, 1), np.float32)
    # additive causal mask for a diagonal block in [k, q] layout
    mask = np.where(np.arange(P)[:, None] > np.arange(P)[None, :], NEG,
                    0.0).astype(np.float32)
    ident = np.eye(P, dtype=np.float32)
    return cos_t, sin_t, perm, ones, mask, ident


def kernel(x, position_ids, Wq, Wk, Wv, Wo):
    x = np.asarray(x, dtype=np.float32)
    Wq = np.asarray(Wq, dtype=np.float32)
    Wk = np.asarray(Wk, dtype=np.float32)
    Wv = np.asarray(Wv, dtype=np.float32)
    Wo = np.asarray(Wo, dtype=np.float32)
    cos_t, sin_t, perm, ones, mask, ident = _host_tables(position_ids)

    in_maps = []
    for c in range(NCORES):
        b, g = c // NKV, c % NKV
        in_maps.append({
            "xT": np.ascontiguousarray(x[b].T),
            "wq": np.ascontiguousarray(Wq[:, g * DLOC:(g + 1) * DLOC]),
            "wk": np.ascontiguousarray(Wk[:, g * HD:(g + 1) * HD]),
            "wv": np.ascontiguousarray(Wv[:, g * HD:(g + 1) * HD]),
            "wo": np.ascontiguousarray(Wo[g * DLOC:(g + 1) * DLOC, :]),
            "cos": cos_t, "sin": sin_t, "perm": perm, "ones": ones,
            "maskneg": mask, "ident": ident,
        })

    nc = _get_program()
    res = run_bass_kernel_spmd(nc, in_maps, core_ids=list(range(NCORES)))

    out = np.zeros((B, S, H), np.float32)
    for c in range(NCORES):
        out[c // NKV] += res.results[c]["yT"].T
    return out


# revision 17
# speedup vs baseline: 1.4885x; 1.2973x over previous
"""Causal self-attention (GQA + RoPE) TRN2 Bass kernel, 8-way sharded.

Sharding: core c -> batch b = c//4, KV group g = c%4 (Q heads 4g..4g+3).
Each core computes its 4 Q heads' attention for its batch plus the partial
output projection (rows of Wo for those heads). Host sums the 4 partials
per batch and transposes back to [S, H].

Device layouts are transposed ([feature, seq]) so every matmul contracts
along the partition dim. All matmuls run in float32r (TF32) at full PE
rate; softmax runs in fp32 without max-subtraction (logits are O(5) for
this distribution, exp is safe).
"""
import sys
sys.path.insert(0, '/opt/trn_rl_repo')
import numpy as np

import concourse.bacc as bacc
import concourse.tile as tile
from concourse import mybir
from concourse.bass_utils import run_bass_kernel_spmd

F32 = mybir.dt.float32
F32R = mybir.dt.float32r

B, S, H = 2, 2048, 2048
NH, NKV, HD = 16, 4, 128
ROPE_BASE = 10000.0
SCALE = 1.0 / float(np.sqrt(HD))
NCORES = 8
HEADS_PER_CORE = NH // NKV          # 4 Q heads per KV group
DLOC = HEADS_PER_CORE * HD          # 512 local head dims
P = 128                             # partitions
NQC = S // 512                      # 4 query chunks of 512
NKB = S // P                        # 16 key blocks of 128
NHT = H // P                        # 16 contraction tiles of 128
NEG = -1.0e30

_CACHED_NC = None


def _build_program():
    """Emit the SPMD per-core program (identical on all 8 cores)."""
    nc = bacc.Bacc("TRN2", target_bir_lowering=False, debug=False)

    d_xT = nc.dram_tensor("xT", [H, S], F32R, kind="ExternalInput").ap()
    d_wq = nc.dram_tensor("wq", [H, DLOC], F32R, kind="ExternalInput").ap()
    d_wk = nc.dram_tensor("wk", [H, HD], F32R, kind="ExternalInput").ap()
    d_wv = nc.dram_tensor("wv", [H, HD], F32R, kind="ExternalInput").ap()
    d_wo = nc.dram_tensor("wo", [DLOC, H], F32R, kind="ExternalInput").ap()
    d_cos = nc.dram_tensor("cos", [P, S], F32, kind="ExternalInput").ap()
    d_sin = nc.dram_tensor("sin", [P, S], F32, kind="ExternalInput").ap()
    d_perm = nc.dram_tensor("perm", [P, P], F32R, kind="ExternalInput").ap()
    d_ones = nc.dram_tensor("ones", [P, 1], F32R, kind="ExternalInput").ap()
    d_mask = nc.dram_tensor("maskneg", [P, P], F32, kind="ExternalInput").ap()
    d_id = nc.dram_tensor("ident", [P, P], F32R, kind="ExternalInput").ap()
    d_yT = nc.dram_tensor("yT", [H, S], F32, kind="ExternalOutput").ap()

    with tile.TileContext(nc) as tc:
        with tc.tile_pool(name="persist", bufs=1) as pers:
            # Persistent SBUF tensors: roped Q^T/K^T, V ([s,d] blocks),
            # unnormalized-then-normalized attention output A.
            qT = [pers.tile([P, S], F32R, tag=f"qT{h}", name=f"qT{h}") for h in range(HEADS_PER_CORE)]
            kT = pers.tile([P, S], F32R, tag="kT")
            vblk = [pers.tile([P, HD], F32R, tag=f"v{sb}", name=f"v{sb}") for sb in range(NKB)]
            aT = [pers.tile([P, S], F32R, tag=f"aT{h}", name=f"aT{h}") for h in range(HEADS_PER_CORE)]

            # ---------------- Phase 1: projections + RoPE -----------------
            with tc.tile_pool(name="p1sb", bufs=1) as p1, \
                 tc.tile_pool(name="p1x", bufs=2) as p1x, \
                 tc.tile_pool(name="p1w", bufs=1) as p1w, \
                 tc.tile_pool(name="p1st", bufs=2) as p1s, \
                 tc.tile_pool(name="p1ps", bufs=1, space="PSUM") as pp, \
                 tc.tile_pool(name="p1ps2", bufs=1, space="PSUM") as pp2:
                t_cos = p1.tile([P, S], F32, tag="cos")
                t_sin = p1.tile([P, S], F32, tag="sin")
                t_perm = p1.tile([P, P], F32R, tag="perm")
                t_id = p1.tile([P, P], F32R, tag="ident")

                wqts = [None] * NHT
                wkts = [None] * NHT
                wvts = [None] * NHT

                def emit_rope(qt):
                    """RoPE + V-transpose for quarter qt (reads staged
                    pre/vts SBUF tiles, not the PSUM accumulators)."""
                    s0 = qt * 512
                    pres, vt_s = staged[qt]
                    for c in range(5):
                        dst = qT[c] if c < HEADS_PER_CORE else kT
                        pre = pres[c]
                        rot = pp2.tile([P, 512], F32, tag="rot",
                                       name=f"rot{qt}_{c}")
                        nc.tensor.matmul(rot[:], t_perm[:], pre[:],
                                         start=True, stop=True)
                        t1 = p1s.tile([P, 512], F32, tag="t1",
                                      name=f"t1_{qt}_{c}")
                        nc.vector.tensor_mul(t1[:], pre[:].bitcast(F32),
                                             t_cos[:, s0:s0 + 512])
                        t2 = p1s.tile([P, 512], F32, tag="t2",
                                      name=f"t2_{qt}_{c}")
                        nc.vector.tensor_mul(t2[:], rot[:], t_sin[:, s0:s0 + 512])
                        nc.vector.tensor_add(dst[:, s0:s0 + 512], t1[:], t2[:])
                    for j in range(4):
                        ps_vt = pp2.tile([P, P], F32R, tag="psvt",
                                         name=f"psvt{qt}_{j}")
                        nc.tensor.transpose(ps_vt[:], vt_s[:, j * P:(j + 1) * P],
                                            t_id[:])
                        nc.scalar.copy(out=vblk[qt * 4 + j][:],
                                       in_=ps_vt[:].bitcast(F32))

                staged = {}
                for qt in range(4):               # s quarters of 512
                    s0 = qt * 512
                    # 6 concurrent PSUM accumulators: 4 Q chunks, K, V
                    accs = [pp.tile([P, 512], F32, tag=f"acc{c}",
                                    name=f"pacc{qt}_{c}") for c in range(6)]
                    for ht in range(NHT):
                        # interleave input DMAs with compute so the first
                        # matmul starts after ~2 DMAs, not a full-quarter fill
                        xt = p1x.tile([P, 512], F32R, tag="xt", bufs=6,
                                      name=f"xt{qt}_{ht}")
                        nc.sync.dma_start(out=xt[:],
                                          in_=d_xT[ht * P:(ht + 1) * P, s0:s0 + 512])
                        if qt == 0:               # weights resident, loaded once
                            wqts[ht] = p1w.tile([P, DLOC], F32R, tag=f"wq{ht}",
                                                name=f"wq{ht}")
                            nc.sync.dma_start(out=wqts[ht][:],
                                              in_=d_wq[ht * P:(ht + 1) * P, :])
                            wkts[ht] = p1w.tile([P, HD], F32R, tag=f"wk{ht}",
                                                name=f"wk{ht}")
                            nc.sync.dma_start(out=wkts[ht][:],
                                              in_=d_wk[ht * P:(ht + 1) * P, :])
                            wvts[ht] = p1w.tile([P, HD], F32R, tag=f"wv{ht}",
                                                name=f"wv{ht}")
                            nc.sync.dma_start(out=wvts[ht][:],
                                              in_=d_wv[ht * P:(ht + 1) * P, :])
                        st = (ht == 0)
                        sp = (ht == NHT - 1)
                        for c in range(HEADS_PER_CORE):
                            nc.tensor.matmul(accs[c][:],
                                             wqts[ht][:, c * P:(c + 1) * P], xt[:],
                                             start=st, stop=sp)
                        nc.tensor.matmul(accs[4][:], wkts[ht][:], xt[:],
                                         start=st, stop=sp)
                        nc.tensor.matmul(accs[5][:], wvts[ht][:], xt[:],
                                         start=st, stop=sp)
                        if qt == 0 and ht == 1:
                            # RoPE tables arrive behind the first compute wave
                            nc.sync.dma_start(out=t_cos[:], in_=d_cos)
                            nc.sync.dma_start(out=t_sin[:], in_=d_sin)
                            nc.sync.dma_start(out=t_perm[:], in_=d_perm)
                            nc.sync.dma_start(out=t_id[:], in_=d_id)
                        if ht == 2 and qt > 0:
                            # previous quarter's RoPE, emitted behind this
                            # quarter's first matmul wave so the PE never
                            # waits on the ACT copy chain
                            emit_rope(qt - 1)

                    # stage the accumulators out to SBUF (frees PSUM for the
                    # next quarter); rot/rope consume these staged tiles later
                    pres = []
                    for c in range(5):
                        pre = p1s.tile([P, 512], F32R, tag="pre", bufs=12,
                                       name=f"pre{qt}_{c}")
                        nc.scalar.copy(out=pre[:], in_=accs[c][:])
                        pres.append(pre)
                    vt_s = p1s.tile([P, 512], F32R, tag="vts", bufs=3,
                                    name=f"vts{qt}")
                    nc.scalar.copy(out=vt_s[:], in_=accs[5][:])
                    staged[qt] = (pres, vt_s)
                emit_rope(3)

            # Prefetch phase-3 weights early so the output projection never
            # waits on DMA (8 MB takes ~25 us).
            wot = []
            with tc.tile_pool(name="p3w", bufs=1) as p3w:
                for dinb in range(HEADS_PER_CORE):
                    wt = p3w.tile([P, H], F32R, tag=f"wo{dinb}", name=f"wo{dinb}")
                    nc.sync.dma_start(out=wt[:],
                                      in_=d_wo[dinb * P:(dinb + 1) * P, :])
                    wot.append(wt)

                # -------------- Phase 2: causal attention -----------------
                # Software-pipelined: scores/exp for block kb+1 are emitted
                # before attnV/sums of block kb, so the PE always has an
                # independent matmul to run while ACT computes the exp.
                # Row sums accumulate in a parallel PSUM chain of
                # ones-matmuls (no DVE reduction on the critical path).
                with tc.tile_pool(name="p2sb", bufs=1) as p2, \
                     tc.tile_pool(name="p2pt", bufs=6) as p2pt, \
                     tc.tile_pool(name="p2acc", bufs=2) as p2a, \
                     tc.tile_pool(name="p2ps", bufs=4, space="PSUM") as psc, \
                     tc.tile_pool(name="p2pat", bufs=2, space="PSUM") as pat, \
                     tc.tile_pool(name="p2psum", bufs=2, space="PSUM") as psums:
                    t_mask = p2.tile([P, P], F32, tag="mask")
                    t_ones = p2.tile([P, 1], F32R, tag="ones")
                    nc.sync.dma_start(out=t_mask[:], in_=d_mask)
                    nc.sync.dma_start(out=t_ones[:], in_=d_ones)

                    for h in range(HEADS_PER_CORE):
                        for qc in range(NQC):
                            q0 = qc * 512
                            nkb = 4 * qc + 4      # key blocks 0..nkb-1
                            attn = pat.tile([P, 512], F32, tag="attn",
                                            name=f"attn_{h}_{qc}")
                            ps_sm = psums.tile([1, 512], F32, tag="ps_sm",
                                               name=f"ps_sm_{h}_{qc}")

                            def scores_exp(kb):
                                j = kb - 4 * qc   # >=0 on the diagonal
                                off = j * P if j >= 0 else 0
                                w = 512 - off
                                ps_s = psc.tile([P, 512], F32, tag="ps_s",
                                                name=f"ps_{h}_{qc}_{kb}")
                                nc.tensor.matmul(ps_s[:, :w],
                                                 kT[:, kb * P:(kb + 1) * P],
                                                 qT[h][:, q0 + off:q0 + 512],
                                                 start=True, stop=True)
                                if j >= 0:
                                    nc.vector.tensor_add(ps_s[:, 0:P],
                                                         ps_s[:, 0:P], t_mask[:])
                                pt = p2pt.tile([P, 512], F32R, tag="pt",
                                               name=f"pt_{h}_{qc}_{kb}")
                                nc.scalar.activation(
                                    out=pt[:, :w], in_=ps_s[:, :w],
                                    func=mybir.ActivationFunctionType.Exp,
                                    scale=SCALE)
                                return pt, off, w

                            def consume(kb, pt, off, w):
                                nc.tensor.matmul(attn[:, off:512], vblk[kb][:],
                                                 pt[:, :w],
                                                 start=(kb == 0),
                                                 stop=(kb == nkb - 1))
                                nc.tensor.matmul(ps_sm[:, off:512], t_ones[:],
                                                 pt[:, :w],
                                                 start=(kb == 0),
                                                 stop=(kb == nkb - 1))

                            prev = None
                            for kb in range(nkb):
                                cur = (kb,) + scores_exp(kb)
                                if prev is not None:
                                    consume(*prev)
                                prev = cur
                            consume(*prev)

                            rec = p2a.tile([1, 512], F32, tag="rec",
                                           name=f"rec_{h}_{qc}")
                            rscr = p2a.tile([1, 512], F32, tag="rscr",
                                            name=f"rscr_{h}_{qc}")
                            nc.vector.reciprocal_approx_accurate(
                                out=rec[:], in_=ps_sm[:], scratch=rscr[:])
                            recb = p2a.tile([P, 512], F32, tag="recb",
                                            name=f"recb_{h}_{qc}")
                            nc.gpsimd.partition_broadcast(recb[:], rec[:])
                            nc.vector.tensor_mul(aT[h][:, q0:q0 + 512],
                                                 attn[:], recb[:])

                # -------------- Phase 3: output projection ----------------
                with tc.tile_pool(name="p3y", bufs=3) as p3y, \
                     tc.tile_pool(name="p3ps", bufs=4, space="PSUM") as pyo:
                    for hc in range(NHT):
                        yt_sb = p3y.tile([P, S], F32, tag="yt")
                        for nch in range(NQC):
                            n0 = nch * 512
                            accy = pyo.tile([P, 512], F32, tag="accy")
                            for dinb in range(HEADS_PER_CORE):
                                nc.tensor.matmul(accy[:],
                                                 wot[dinb][:, hc * P:(hc + 1) * P],
                                                 aT[dinb][:, n0:n0 + 512],
                                                 start=(dinb == 0),
                                                 stop=(dinb == HEADS_PER_CORE - 1))
                            nc.scalar.copy(out=yt_sb[:, n0:n0 + 512], in_=accy[:])
                        nc.sync.dma_start(out=d_yT[hc * P:(hc + 1) * P, :],
                                          in_=yt_sb[:])
    nc.compile()
    return nc


def _get_program():
    global _CACHED_NC
    if _CACHED_NC is None:
        _CACHED_NC = _build_program()
    return _CACHED_NC


def _host_tables(position_ids):
    pos = np.asarray(position_ids).reshape(-1).astype(np.float64)  # [S]
    half = HD // 2
    inv = 1.0 / (ROPE_BASE ** (np.arange(half, dtype=np.float64) * 2.0 / HD))
    ang = pos[:, None] * inv[None, :]                 # [S, 64]
    cos = np.cos(ang).T.astype(np.float32)            # [64, S]
    sin = np.sin(ang).T.astype(np.float32)
    cos_t = np.ascontiguousarray(np.concatenate([cos, cos], axis=0))  # [128, S]
    sin_t = np.ascontiguousarray(np.concatenate([sin, sin], axis=0))
    # perm implements rotate_half in [d, s] layout, signs folded in:
    # out[d'] = -in[d'+64] (d'<64), +in[d'-64] (d'>=64); lhsT[d, d'] layout.
    perm = np.zeros((P, P), np.float32)
    for dp in range(half):
        perm[dp + half, dp] = -1.0
    for dp in range(half, HD):
        perm[dp - half, dp] = 1.0
    ones = np.ones((P, 1), np.float32)
    # additive causal mask for a diagonal block in [k, q] layout
    mask = np.where(np.arange(P)[:, None] > np.arange(P)[None, :], NEG,
                    0.0).astype(np.float32)
    ident = np.eye(P, dtype=np.float32)
    return cos_t, sin_t, perm, ones, mask, ident


def kernel(x, position_ids, Wq, Wk, Wv, Wo):
    x = np.asarray(x, dtype=np.float32)
    Wq = np.asarray(Wq, dtype=np.float32)
    Wk = np.asarray(Wk, dtype=np.float32)
    Wv = np.asarray(Wv, dtype=np.float32)
    Wo = np.asarray(Wo, dtype=np.float32)
    cos_t, sin_t, perm, ones, mask, ident = _host_tables(position_ids)

    in_maps = []
    for c in range(NCORES):
        b, g = c // NKV, c % NKV
        in_maps.append({
            "xT": np.ascontiguousarray(x[b].T),
            "wq": np.ascontiguousarray(Wq[:, g * DLOC:(g + 1) * DLOC]),
            "wk": np.ascontiguousarray(Wk[:, g * HD:(g + 1) * HD]),
            "wv": np.ascontiguousarray(Wv[:, g * HD:(g + 1) * HD]),
            "wo": np.ascontiguousarray(Wo[g * DLOC:(g + 1) * DLOC, :]),
            "cos": cos_t, "sin": sin_t, "perm": perm, "ones": ones,
            "maskneg": mask, "ident": ident,
        })

    nc = _get_program()
    res = run_bass_kernel_spmd(nc, in_maps, core_ids=list(range(NCORES)))

    out = np.zeros((B, S, H), np.float32)
    for c in range(NCORES):
        out[c // NKV] += res.results[c]["yT"].T
    return out


# revision 21
# speedup vs baseline: 1.5066x; 1.0121x over previous
"""Causal self-attention (GQA + RoPE) TRN2 Bass kernel, 8-way sharded.

Sharding: core c -> batch b = c//4, KV group g = c%4 (Q heads 4g..4g+3).
Each core computes its 4 Q heads' attention for its batch plus the partial
output projection (rows of Wo for those heads). Host sums the 4 partials
per batch and transposes back to [S, H].

Device layouts are transposed ([feature, seq]) so every matmul contracts
along the partition dim. All matmuls run in float32r (TF32) at full PE
rate; softmax runs in fp32 without max-subtraction (logits are O(5) for
this distribution, exp is safe).
"""
import sys
sys.path.insert(0, '/opt/trn_rl_repo')
import numpy as np

import concourse.bacc as bacc
import concourse.tile as tile
from concourse import mybir
from concourse.bass_utils import run_bass_kernel_spmd

F32 = mybir.dt.float32
F32R = mybir.dt.float32r

B, S, H = 2, 2048, 2048
NH, NKV, HD = 16, 4, 128
ROPE_BASE = 10000.0
SCALE = 1.0 / float(np.sqrt(HD))
NCORES = 8
HEADS_PER_CORE = NH // NKV          # 4 Q heads per KV group
DLOC = HEADS_PER_CORE * HD          # 512 local head dims
P = 128                             # partitions
NQC = S // 512                      # 4 query chunks of 512
NKB = S // P                        # 16 key blocks of 128
NHT = H // P                        # 16 contraction tiles of 128
NEG = -1.0e30

_CACHED_NC = None


def _build_program():
    """Emit the SPMD per-core program (identical on all 8 cores)."""
    nc = bacc.Bacc("TRN2", target_bir_lowering=False, debug=False)

    d_xT = nc.dram_tensor("xT", [H, S], F32R, kind="ExternalInput").ap()
    d_wq = nc.dram_tensor("wq", [H, DLOC], F32R, kind="ExternalInput").ap()
    d_wkv = nc.dram_tensor("wkv", [H, 2 * HD], F32R, kind="ExternalInput").ap()
    d_wo = nc.dram_tensor("wo", [DLOC, H], F32R, kind="ExternalInput").ap()
    d_cos = nc.dram_tensor("cos", [P, S], F32, kind="ExternalInput").ap()
    d_sin = nc.dram_tensor("sin", [P, S], F32, kind="ExternalInput").ap()
    d_perm = nc.dram_tensor("perm", [P, P], F32R, kind="ExternalInput").ap()
    d_ones = nc.dram_tensor("ones", [P, 1], F32R, kind="ExternalInput").ap()
    d_mask = nc.dram_tensor("maskneg", [P, P], F32, kind="ExternalInput").ap()
    d_id = nc.dram_tensor("ident", [P, P], F32R, kind="ExternalInput").ap()
    d_yT = nc.dram_tensor("yT", [H, S], F32, kind="ExternalOutput").ap()

    with tile.TileContext(nc) as tc:
        with tc.tile_pool(name="persist", bufs=1) as pers:
            # Persistent SBUF tensors: roped Q^T/K^T, V ([s,d] blocks),
            # unnormalized-then-normalized attention output A.
            qT = [pers.tile([P, S], F32R, tag=f"qT{h}", name=f"qT{h}") for h in range(HEADS_PER_CORE)]
            kT = pers.tile([P, S], F32R, tag="kT")
            vblk = [pers.tile([P, HD], F32R, tag=f"v{sb}", name=f"v{sb}") for sb in range(NKB)]
            aT = [pers.tile([P, S], F32R, tag=f"aT{h}", name=f"aT{h}") for h in range(HEADS_PER_CORE)]

            # ---------------- Phase 1: projections + RoPE -----------------
            with tc.tile_pool(name="p1sb", bufs=1) as p1, \
                 tc.tile_pool(name="p1x", bufs=2) as p1x, \
                 tc.tile_pool(name="p1w", bufs=1) as p1w, \
                 tc.tile_pool(name="p1st", bufs=2) as p1s, \
                 tc.tile_pool(name="p1ps", bufs=1, space="PSUM") as pp, \
                 tc.tile_pool(name="p1ps2", bufs=1, space="PSUM") as pp2:
                t_cos = p1.tile([P, S], F32, tag="cos")
                t_sin = p1.tile([P, S], F32, tag="sin")
                t_perm = p1.tile([P, P], F32R, tag="perm")
                t_id = p1.tile([P, P], F32R, tag="ident")

                wqts = [None] * NHT
                wkvts = [None] * NHT

                def emit_rope(qt):
                    """RoPE + V-transpose for quarter qt (reads staged
                    pre/vts SBUF tiles, not the PSUM accumulators)."""
                    s0 = qt * 512
                    pres, vt_s = staged[qt]
                    for c in range(5):
                        dst = qT[c] if c < HEADS_PER_CORE else kT
                        pre = pres[c]
                        rot = pp2.tile([P, 512], F32, tag="rot",
                                       name=f"rot{qt}_{c}")
                        nc.tensor.matmul(rot[:], t_perm[:], pre[:],
                                         start=True, stop=True)
                        t1 = p1s.tile([P, 512], F32, tag="t1",
                                      name=f"t1_{qt}_{c}")
                        nc.vector.tensor_mul(t1[:], pre[:].bitcast(F32),
                                             t_cos[:, s0:s0 + 512])
                        t2 = p1s.tile([P, 512], F32, tag="t2",
                                      name=f"t2_{qt}_{c}")
                        nc.vector.tensor_mul(t2[:], rot[:], t_sin[:, s0:s0 + 512])
                        nc.vector.tensor_add(dst[:, s0:s0 + 512], t1[:], t2[:])
                    for j in range(4):
                        ps_vt = pp2.tile([P, P], F32R, tag="psvt",
                                         name=f"psvt{qt}_{j}")
                        nc.tensor.transpose(ps_vt[:], vt_s[:, j * P:(j + 1) * P],
                                            t_id[:])
                        nc.scalar.copy(out=vblk[qt * 4 + j][:],
                                       in_=ps_vt[:].bitcast(F32))

                staged = {}
                for qt in range(4):               # s quarters of 512
                    s0 = qt * 512
                    # 6 concurrent PSUM accumulators: 4 Q chunks, K, V
                    accs = [pp.tile([P, 512], F32, tag=f"acc{c}",
                                    name=f"pacc{qt}_{c}") for c in range(6)]
                    for ht in range(NHT):
                        # interleave input DMAs with compute so the first
                        # matmul starts after ~2 DMAs, not a full-quarter fill
                        xt = p1x.tile([P, 512], F32R, tag="xt", bufs=6,
                                      name=f"xt{qt}_{ht}")
                        nc.sync.dma_start(out=xt[:],
                                          in_=d_xT[ht * P:(ht + 1) * P, s0:s0 + 512])
                        if qt == 0:               # weights resident, loaded once
                            wqts[ht] = p1w.tile([P, DLOC], F32R, tag=f"wq{ht}",
                                                name=f"wq{ht}")
                            nc.sync.dma_start(out=wqts[ht][:],
                                              in_=d_wq[ht * P:(ht + 1) * P, :])
                            wkvts[ht] = p1w.tile([P, 2 * HD], F32R,
                                                 tag=f"wkv{ht}", name=f"wkv{ht}")
                            nc.sync.dma_start(out=wkvts[ht][:],
                                              in_=d_wkv[ht * P:(ht + 1) * P, :])
                        st = (ht == 0)
                        sp = (ht == NHT - 1)
                        for c in range(HEADS_PER_CORE):
                            nc.tensor.matmul(accs[c][:],
                                             wqts[ht][:, c * P:(c + 1) * P], xt[:],
                                             start=st, stop=sp)
                        nc.tensor.matmul(accs[4][:], wkvts[ht][:, 0:HD], xt[:],
                                         start=st, stop=sp)
                        nc.tensor.matmul(accs[5][:], wkvts[ht][:, HD:2 * HD],
                                         xt[:], start=st, stop=sp)
                        if qt == 0 and ht == 1:
                            # RoPE tables arrive behind the first compute wave
                            nc.sync.dma_start(out=t_cos[:], in_=d_cos)
                            nc.sync.dma_start(out=t_sin[:], in_=d_sin)
                            nc.sync.dma_start(out=t_perm[:], in_=d_perm)
                            nc.sync.dma_start(out=t_id[:], in_=d_id)
                        if ht == 2 and qt > 0:
                            # previous quarter's RoPE, emitted behind this
                            # quarter's first matmul wave so the PE never
                            # waits on the ACT copy chain
                            emit_rope(qt - 1)

                    # stage the accumulators out to SBUF (frees PSUM for the
                    # next quarter); rot/rope consume these staged tiles later
                    pres = []
                    for c in range(5):
                        pre = p1s.tile([P, 512], F32R, tag="pre", bufs=12,
                                       name=f"pre{qt}_{c}")
                        nc.scalar.copy(out=pre[:], in_=accs[c][:])
                        pres.append(pre)
                    vt_s = p1s.tile([P, 512], F32R, tag="vts", bufs=3,
                                    name=f"vts{qt}")
                    nc.scalar.copy(out=vt_s[:], in_=accs[5][:])
                    staged[qt] = (pres, vt_s)
                emit_rope(3)

            # Prefetch phase-3 weights early so the output projection never
            # waits on DMA (8 MB takes ~25 us).
            wot = []
            with tc.tile_pool(name="p3w", bufs=1) as p3w:
                for dinb in range(HEADS_PER_CORE):
                    wt = p3w.tile([P, H], F32R, tag=f"wo{dinb}", name=f"wo{dinb}")
                    nc.sync.dma_start(out=wt[:],
                                      in_=d_wo[dinb * P:(dinb + 1) * P, :])
                    wot.append(wt)

                # -------------- Phase 2: causal attention -----------------
                # Software-pipelined: scores/exp for block kb+1 are emitted
                # before attnV/sums of block kb, so the PE always has an
                # independent matmul to run while ACT computes the exp.
                # Row sums accumulate in a parallel PSUM chain of
                # ones-matmuls (no DVE reduction on the critical path).
                with tc.tile_pool(name="p2sb", bufs=1) as p2, \
                     tc.tile_pool(name="p2pt", bufs=6) as p2pt, \
                     tc.tile_pool(name="p2acc", bufs=2) as p2a, \
                     tc.tile_pool(name="p3y", bufs=4) as p3y, \
                     tc.tile_pool(name="p2ps", bufs=3, space="PSUM") as psc, \
                     tc.tile_pool(name="p2pat", bufs=2, space="PSUM") as pat, \
                     tc.tile_pool(name="p2psum", bufs=1, space="PSUM") as psums, \
                     tc.tile_pool(name="p3ps", bufs=2, space="PSUM") as pyo:
                    t_mask = p2.tile([P, P], F32, tag="mask")
                    t_ones = p2.tile([P, 1], F32R, tag="ones")
                    nc.sync.dma_start(out=t_mask[:], in_=d_mask)
                    nc.sync.dma_start(out=t_ones[:], in_=d_ones)

                    for qc in range(NQC):
                        for h in range(HEADS_PER_CORE):
                            q0 = qc * 512
                            nkb = 4 * qc + 4      # key blocks 0..nkb-1
                            attn = pat.tile([P, 512], F32, tag="attn",
                                            name=f"attn_{h}_{qc}")
                            ps_sm = psums.tile([1, 512], F32, tag="ps_sm",
                                               name=f"ps_sm_{h}_{qc}")

                            def scores_exp(kb):
                                j = kb - 4 * qc   # >=0 on the diagonal
                                off = j * P if j >= 0 else 0
                                w = 512 - off
                                ps_s = psc.tile([P, 512], F32, tag="ps_s",
                                                name=f"ps_{h}_{qc}_{kb}")
                                nc.tensor.matmul(ps_s[:, :w],
                                                 kT[:, kb * P:(kb + 1) * P],
                                                 qT[h][:, q0 + off:q0 + 512],
                                                 start=True, stop=True)
                                if j >= 0:
                                    nc.vector.tensor_add(ps_s[:, 0:P],
                                                         ps_s[:, 0:P], t_mask[:])
                                pt = p2pt.tile([P, 512], F32R, tag="pt",
                                               name=f"pt_{h}_{qc}_{kb}")
                                nc.scalar.activation(
                                    out=pt[:, :w], in_=ps_s[:, :w],
                                    func=mybir.ActivationFunctionType.Exp,
                                    scale=SCALE)
                                return pt, off, w

                            def consume(kb, pt, off, w):
                                nc.tensor.matmul(attn[:, off:512], vblk[kb][:],
                                                 pt[:, :w],
                                                 start=(kb == 0),
                                                 stop=(kb == nkb - 1))
                                nc.tensor.matmul(ps_sm[:, off:512], t_ones[:],
                                                 pt[:, :w],
                                                 start=(kb == 0),
                                                 stop=(kb == nkb - 1))

                            prev = None
                            for kb in range(nkb):
                                cur = (kb,) + scores_exp(kb)
                                if prev is not None:
                                    consume(*prev)
                                prev = cur
                            consume(*prev)

                            rec = p2a.tile([1, 512], F32, tag="rec",
                                           name=f"rec_{h}_{qc}")
                            rscr = p2a.tile([1, 512], F32, tag="rscr",
                                            name=f"rscr_{h}_{qc}")
                            nc.vector.reciprocal_approx_accurate(
                                out=rec[:], in_=ps_sm[:], scratch=rscr[:])
                            recb = p2a.tile([P, 512], F32, tag="recb",
                                            name=f"recb_{h}_{qc}")
                            nc.gpsimd.partition_broadcast(recb[:], rec[:])
                            nc.vector.tensor_mul(aT[h][:, q0:q0 + 512],
                                                 attn[:], recb[:])

                        # ---- Phase 3 slice for this q-chunk: the output
                        # projection over q-range qc needs only aT[*][:, qc],
                        # so it pipelines behind the attention of chunk qc.
                        n0 = qc * 512
                        for hc in range(NHT):
                            accy = pyo.tile([P, 512], F32, tag="accy",
                                            name=f"accy_{qc}_{hc}")
                            for dinb in range(HEADS_PER_CORE):
                                nc.tensor.matmul(accy[:],
                                                 wot[dinb][:, hc * P:(hc + 1) * P],
                                                 aT[dinb][:, n0:n0 + 512],
                                                 start=(dinb == 0),
                                                 stop=(dinb == HEADS_PER_CORE - 1))
                            yt_sb = p3y.tile([P, 512], F32, tag="yt",
                                             name=f"yt_{qc}_{hc}")
                            nc.scalar.copy(out=yt_sb[:], in_=accy[:])
                            nc.sync.dma_start(
                                out=d_yT[hc * P:(hc + 1) * P, n0:n0 + 512],
                                in_=yt_sb[:])
    nc.compile()
    return nc


def _get_program():
    global _CACHED_NC
    if _CACHED_NC is None:
        _CACHED_NC = _build_program()
    return _CACHED_NC


def _host_tables(position_ids):
    pos = np.asarray(position_ids).reshape(-1).astype(np.float64)  # [S]
    half = HD // 2
    inv = 1.0 / (ROPE_BASE ** (np.arange(half, dtype=np.float64) * 2.0 / HD))
    ang = pos[:, None] * inv[None, :]                 # [S, 64]
    cos = np.cos(ang).T.astype(np.float32)            # [64, S]
    sin = np.sin(ang).T.astype(np.float32)
    cos_t = np.ascontiguousarray(np.concatenate([cos, cos], axis=0))  # [128, S]
    sin_t = np.ascontiguousarray(np.concatenate([sin, sin], axis=0))
    # perm implements rotate_half in [d, s] layout, signs folded in:
    # out[d'] = -in[d'+64] (d'<64), +in[d'-64] (d'>=64); lhsT[d, d'] layout.
    perm = np.zeros((P, P), np.float32)
    for dp in range(half):
        perm[dp + half, dp] = -1.0
    for dp in range(half, HD):
        perm[dp - half, dp] = 1.0
    ones = np.ones((P, 1), np.float32)
    # additive causal mask for a diagonal block in [k, q] layout
    mask = np.where(np.arange(P)[:, None] > np.arange(P)[None, :], NEG,
                    0.0).astype(np.float32)
    ident = np.eye(P, dtype=np.float32)
    return cos_t, sin_t, perm, ones, mask, ident


def kernel(x, position_ids, Wq, Wk, Wv, Wo):
    x = np.asarray(x, dtype=np.float32)
    Wq = np.asarray(Wq, dtype=np.float32)
    Wk = np.asarray(Wk, dtype=np.float32)
    Wv = np.asarray(Wv, dtype=np.float32)
    Wo = np.asarray(Wo, dtype=np.float32)
    cos_t, sin_t, perm, ones, mask, ident = _host_tables(position_ids)

    in_maps = []
    for c in range(NCORES):
        b, g = c // NKV, c % NKV
        in_maps.append({
            "xT": np.ascontiguousarray(x[b].T),
            "wq": np.ascontiguousarray(Wq[:, g * DLOC:(g + 1) * DLOC]),
            "wkv": np.ascontiguousarray(np.concatenate(
                [Wk[:, g * HD:(g + 1) * HD], Wv[:, g * HD:(g + 1) * HD]],
                axis=1)),
            "wo": np.ascontiguousarray(Wo[g * DLOC:(g + 1) * DLOC, :]),
            "cos": cos_t, "sin": sin_t, "perm": perm, "ones": ones,
            "maskneg": mask, "ident": ident,
        })

    nc = _get_program()
    res = run_bass_kernel_spmd(nc, in_maps, core_ids=list(range(NCORES)))

    out = np.zeros((B, S, H), np.float32)
    for c in range(NCORES):
        out[c // NKV] += res.results[c]["yT"].T
    return out
